# revision 11
# baseline (speedup 1.0000x reference)
"""Trainium2 Bass kernel for nn_CrossAttn_23888608100978 (retrieval_knn).

Math (see reference):
  support = X[:S]; query = X[S:]
  sim = support @ codebook.T          [S, C]
  top = argmax(sim, 1); code_n = codebook[top]
  proto = mean(support); proto_code = mean(code_n)
  out = (||q - proto|| + ||q - proto_code||) / 2

Key transform: proto_code only needs the MEAN of gathered codebook rows, so
argmax+gather is replaced by a one-hot row-max mask whose per-code counts
(histogram) are accumulated on-chip; code_sum = counts @ codebook.

Sharding: 8 cores, each takes S/8 support rows + Q/8 query rows; codebook is
replicated. A single [1,1025] AllReduce combines (proto_sum, code_sum, count).
"""

import sys

sys.path.insert(0, "/opt/trn_rl_repo")

import numpy as np

import concourse.bass as bass
import concourse.tile as tile
from concourse import bacc, mybir
from concourse.bass_utils import run_bass_kernel_spmd
from concourse.masks import make_identity

# Problem constants (hardcoded per contract)
N, D, C, S = 100000, 512, 4096, 50000
M = 8  # cores
SH = S // M  # 6250 support rows per core
QH = (N - S) // M  # 6250 query rows per core
P = 128
NT = (SH + P - 1) // P  # 49 tiles
LAST = SH - (NT - 1) * P  # 106 rows in the ragged last tile
NQT = (QH + P - 1) // P
QLAST = QH - (NQT - 1) * P
NCB = C // P  # 32 codebook row-blocks
KD = D // P  # 4 contraction chunks

f32 = mybir.dt.float32
bf16 = mybir.dt.bfloat16
f16 = mybir.dt.float16
f8 = mybir.dt.float8e4

Alu = mybir.AluOpType
Act = mybir.ActivationFunctionType

LAST_EXEC_NS = None
_CACHE = {}


def _build_nc():
    nc = bacc.Bacc(trn_type="TRN2", num_devices=M)
    xs = nc.dram_tensor("xs", [SH, D], f32, kind="ExternalInput")
    xq = nc.dram_tensor("xq", [QH, D], f32, kind="ExternalInput")
    cb = nc.dram_tensor("cb", [C, D], f32, kind="ExternalInput")
    out = nc.dram_tensor("out", [SH], f32, kind="ExternalOutput")

    with tile.TileContext(nc) as tc:
        with (
            tc.tile_pool(name="consts", bufs=1) as cp,
            tc.tile_pool(name="xload", bufs=3) as xp,
            tc.tile_pool(name="xcast", bufs=3) as xbp,
            tc.tile_pool(name="xtsb", bufs=3) as xtp,
            tc.tile_pool(name="simsb", bufs=2) as simp,
            tc.tile_pool(name="small", bufs=2) as smp,
            tc.tile_pool(name="dram", bufs=1, space="DRAM") as dp,
        ):
            id_bf = cp.tile([P, P], bf16)
            make_identity(nc, id_bf[:])
            id_f = cp.tile([P, P], f32)
            make_identity(nc, id_f[:])
            id_f8 = cp.tile([P, P], f8)
            make_identity(nc, id_f8[:])
            ones_col = cp.tile([P, 1], f32)
            nc.gpsimd.memset(ones_col[:], 1.0)
            ones_row = cp.tile([1, P], f32)
            nc.gpsimd.memset(ones_row[:], 1.0)

            ones_h = cp.tile([P, 1], f16)
            nc.gpsimd.memset(ones_h[:], 1.0)
            cbt = cp.tile([P, KD, C], f8)  # codebook^T (d on partitions)
            cbn = cp.tile([P, NCB, D], f32)  # codebook natural (c on partitions)
            acc = cp.tile([P, C], f16)  # one-hot mask accumulator
            nc.gpsimd.memset(acc[:], 0.0)
            acc512 = cp.tile([P, D], f32)  # support row-sum accumulator
            nc.vector.memset(acc512[:], 0.0)
            ss1 = cp.tile([P, NQT], f32)
            nc.vector.memset(ss1[:], 0.0)
            ss2 = cp.tile([P, NQT], f32)
            nc.vector.memset(ss2[:], 0.0)
            counts_sb = cp.tile([P, NCB], f32)
            pro_b = cp.tile([P, D], f32)  # proto broadcast
            proc_b = cp.tile([P, D], f32)  # proto_code broadcast
            cc_sb = cp.tile([1, 2 * D + 1], f32)
            ccr = cp.tile([1, 2 * D + 1], f32)
            proto_sb = cp.tile([1, D], f32)
            protoc_sb = cp.tile([1, D], f32)
            rec = cp.tile([1, 1], f32)
            scoresT = cp.tile([NQT, P], f32)

            # ---- phase 0: codebook load + layouts ----
            with (
                tc.tile_pool(name="psim", bufs=3, space="PSUM") as pmm,
                tc.tile_pool(name="ptr", bufs=2, space="PSUM") as ptr,
            ):
                for j in range(NCB):
                    cstg = xp.tile([P, D], f32, tag="cstg")
                    nc.sync.dma_start(cstg[:], cb[j * P : (j + 1) * P, :])
                    nc.gpsimd.tensor_copy(cbn[:, j, :], cstg[:])
                    cbf = xbp.tile([P, D], bf16, tag="cbf")
                    nc.gpsimd.tensor_copy(cbf[:], cstg[:])
                    tp = ptr.tile([P, KD * P], bf16, tag="tp")
                    for k in range(KD):
                        nc.tensor.transpose(
                            tp[:, k * P : (k + 1) * P],
                            cbf[:, k * P : (k + 1) * P],
                            id_bf[:],
                        )
                    nc.scalar.copy(
                        cbt[:, :, j * P : (j + 1) * P],
                        tp[:].rearrange("p (k c) -> p k c", k=KD),
                    )

                # ---- phase 1: support tiles ----
                for t in range(NT):
                    rows = P if t < NT - 1 else LAST
                    xf = xp.tile([P, D], f32, tag="xf")
                    nc.sync.dma_start(xf[:rows], xs[t * P : t * P + rows, :])
                    xb = xbp.tile([P, D], bf16, tag="xb")
                    nc.gpsimd.tensor_copy(xb[:rows], xf[:rows])
                    tp = ptr.tile([P, KD * P], bf16, tag="tp")
                    for k in range(KD):
                        nc.tensor.transpose(
                            tp[:, k * P : k * P + rows],
                            xb[:rows, k * P : (k + 1) * P],
                            id_bf[:rows, :rows],
                        )
                    xts = xtp.tile([P, KD * P], f8, tag="xts")
                    if rows == P:
                        nc.scalar.copy(xts[:], tp[:])
                    else:
                        for k in range(KD):
                            nc.scalar.copy(
                                xts[:, k * P : k * P + rows],
                                tp[:, k * P : k * P + rows],
                            )

                    sim = simp.tile([P, C], f16, tag="sim")
                    xts3 = xts[:].rearrange("p (k m) -> p k m", k=KD)
                    for q in range(4):
                        ps = pmm.tile([P, 1024], f32, tag="ps")
                        for kp in range(2):
                            for nh in range(2):
                                nc.tensor.matmul(
                                    ps[:rows, nh * 512 : (nh + 1) * 512],
                                    xts3[:, 2 * kp : 2 * kp + 2, :rows],
                                    cbt[:, 2 * kp : 2 * kp + 2, q * 1024 + nh * 512 : q * 1024 + (nh + 1) * 512],
                                    start=(kp == 0),
                                    stop=(kp == 1),
                                    perf_mode=mybir.MatmulPerfMode.DoubleRow,
                                )
                        nc.scalar.copy(
                            sim[:rows, q * 1024 : (q + 1) * 1024], ps[:rows, :]
                        )
                    # row-max via tensor_tensor max tree (2x fp16 DVE mode)
                    mt = smp.tile([P, 2048], f16, tag="mt")
                    mu = smp.tile([P, 1024], f16, tag="mu")
                    nc.vector.tensor_tensor(mt[:rows], sim[:rows, 0:2048], sim[:rows, 2048:4096], op=Alu.max)
                    nc.vector.tensor_tensor(mu[:rows], mt[:rows, 0:1024], mt[:rows, 1024:2048], op=Alu.max)
                    nc.vector.tensor_tensor(mt[:rows, 0:512], mu[:rows, 0:512], mu[:rows, 512:1024], op=Alu.max)
                    nc.vector.tensor_tensor(mu[:rows, 0:256], mt[:rows, 0:256], mt[:rows, 256:512], op=Alu.max)
                    nc.vector.tensor_tensor(mt[:rows, 0:128], mu[:rows, 0:128], mu[:rows, 128:256], op=Alu.max)
                    rm = smp.tile([P, 1], f16, tag="rm")
                    nc.vector.reduce_max(
                        rm[:rows], mt[:rows, 0:128], axis=mybir.AxisListType.X
                    )
                    # acc += (sim >= rowmax)  -- fused one-hot accumulate
                    nc.vector.scalar_tensor_tensor(
                        acc[:rows],
                        sim[:rows],
                        rm[:rows],
                        acc[:rows],
                        op0=Alu.is_ge,
                        op1=Alu.add,
                    )
                    # support row-sum
                    nc.gpsimd.tensor_add(acc512[:rows], acc512[:rows], xf[:rows])

            # ---- phase 2: counts, code_sum, proto_sum, all-reduce ----
            with tc.tile_pool(name="pend", bufs=1, space="PSUM") as pe:
                pcnt = pe.tile([P, NCB], f32, tag="pcnt")
                for j in range(NCB):
                    nc.tensor.matmul(
                        pcnt[:, j : j + 1],
                        acc[:, j * P : (j + 1) * P],
                        ones_h[:],
                        start=True,
                        stop=True,
                    )
                nc.scalar.copy(counts_sb[:], pcnt[:])

                cnt_rs = smp.tile([P, 1], f32, tag="cntrs")
                nc.vector.reduce_sum(
                    cnt_rs[:], counts_sb[:], axis=mybir.AxisListType.X
                )
                pct = pe.tile([1, 1], f32, tag="pct")
                nc.tensor.matmul(pct[:], ones_col[:], cnt_rs[:], start=True, stop=True)

                pcode = pe.tile([1, D], f32, tag="pcode")
                for j in range(NCB):
                    nc.tensor.matmul(
                        pcode[:],
                        counts_sb[:, j : j + 1],
                        cbn[:, j, :],
                        start=(j == 0),
                        stop=(j == NCB - 1),
                    )
                pproto = pe.tile([1, D], f32, tag="pproto")
                nc.tensor.matmul(pproto[:], ones_col[:], acc512[:], start=True, stop=True)

                nc.scalar.copy(cc_sb[:, 0:D], pproto[:])
                nc.scalar.copy(cc_sb[:, D : 2 * D], pcode[:])
                nc.scalar.copy(cc_sb[:, 2 * D : 2 * D + 1], pct[:])

                ccin = dp.tile([1, 2 * D + 1], f32)
                ccout = dp.tile([1, 2 * D + 1], f32)
                nc.sync.dma_start(ccin[:], cc_sb[:])
                nc.gpsimd.collective_compute(
                    "AllReduce",
                    Alu.add,
                    replica_groups=[list(range(M))],
                    ins=[ccin[:].opt()],
                    outs=[ccout[:].opt()],
                )
                nc.sync.dma_start(ccr[:], ccout[:])

                # protos
                nc.scalar.mul(proto_sb[:], ccr[:, 0:D], 1.0 / S)
                nc.vector.reciprocal(rec[:], ccr[:, 2 * D : 2 * D + 1])
                nc.scalar.activation(
                    protoc_sb[:], ccr[:, D : 2 * D], Act.Copy, scale=rec[:]
                )
                # broadcast to 128 partitions
                pb = pe.tile([P, D], f32, tag="pb")
                nc.tensor.matmul(pb[:], ones_row[:], proto_sb[:], start=True, stop=True)
                nc.scalar.copy(pro_b[:], pb[:])
                pb2 = pe.tile([P, D], f32, tag="pb")
                nc.tensor.matmul(pb2[:], ones_row[:], protoc_sb[:], start=True, stop=True)
                nc.scalar.copy(proc_b[:], pb2[:])

                # ---- phase 3: query tiles ----
                for t in range(NQT):
                    rows = P if t < NQT - 1 else QLAST
                    qf = xp.tile([P, D], f32, tag="xf")
                    nc.sync.dma_start(qf[:rows], xq[t * P : t * P + rows, :])
                    d1 = xbp.tile([P, D], f32, tag="d1")
                    nc.gpsimd.tensor_sub(d1[:rows], qf[:rows], pro_b[:rows])
                    nc.scalar.activation(
                        d1[:rows],
                        d1[:rows],
                        Act.Square,
                        accum_out=ss1[:rows, t : t + 1],
                    )
                    d2 = xtp.tile([P, D], f32, tag="d2")
                    nc.vector.tensor_sub(d2[:rows], qf[:rows], proc_b[:rows])
                    nc.scalar.activation(
                        d2[:rows],
                        d2[:rows],
                        Act.Square,
                        accum_out=ss2[:rows, t : t + 1],
                    )

                # ---- epilogue: sqrt, average, transpose, store ----
                nc.scalar.sqrt(ss1[:], ss1[:])
                nc.scalar.sqrt(ss2[:], ss2[:])
                nc.vector.tensor_add(ss1[:], ss1[:], ss2[:])
                nc.scalar.mul(ss1[:], ss1[:], 0.5)
                pst = pe.tile([NQT, P], f32, tag="pst")
                nc.tensor.transpose(pst[:], ss1[:], id_f[:])
                nc.scalar.copy(scoresT[:], pst[:])
                nc.sync.dma_start(
                    out[0 : (NQT - 1) * P].rearrange("(a b) -> a b", b=P),
                    scoresT[: NQT - 1, :],
                )
                nc.sync.dma_start(
                    out[(NQT - 1) * P : SH], scoresT[NQT - 1 : NQT, 0:QLAST]
                )

    nc.finalize()
    return nc


def kernel(X, codebook_sum, prompt_mask, num_support):
    global LAST_EXEC_NS
    X = np.ascontiguousarray(np.asarray(X, dtype=np.float32))
    cb = np.ascontiguousarray(np.asarray(codebook_sum, dtype=np.float32))
    assert int(num_support) == S and X.shape == (N, D) and cb.shape == (C, D)

    if "nc" not in _CACHE:
        _CACHE["nc"] = _build_nc()
    nc = _CACHE["nc"]

    in_maps = []
    for m in range(M):
        in_maps.append(
            {
                "xs": np.ascontiguousarray(X[m * SH : (m + 1) * SH]),
                "xq": np.ascontiguousarray(X[S + m * QH : S + (m + 1) * QH]),
                "cb": cb,
            }
        )
    res = run_bass_kernel_spmd(nc, in_maps, core_ids=list(range(M)))
    LAST_EXEC_NS = res.exec_time_ns
    return np.concatenate([res.results[m]["out"] for m in range(M)]).astype(np.float32)
